# revision 20
# baseline (speedup 1.0000x reference)
"""DeepSeek-style MoE block (SwiGLU experts, top-k routing) on 8 Trainium2 cores.

Expert-parallel sharding: each of the 8 cores owns E/8 = 2 experts and receives
only the tokens routed to those experts (host-side dispatch). The device kernel
computes, per expert e with gathered/padded tokens XT [D, TG] (transposed):

    GT = W0e @ X^T          (PSUM, fp32r matmuls, DFF on partitions)
    UT = W1e @ X^T
    HT = silu(s0*GT) * UT   (SBUF, [DFF, TG])
    Y  = (HT)^T @ W2e^T     (tokens on partitions)
    Yout = coef[token] * Y  where coef = s1*s2*cw  (cw = summed routing weights)

The host then scatter-adds each expert's Yout rows into the dense [T, D] output
(the unshard/combine step for expert-parallel sharding).

Perf notes (from NTFF traces):
 - fp32r matmuls run at 1 cycle/row warm (N>=256), so phase-1 N=TG and
   phase-2 N=512 keep the PE at full rate; plain fp32 would be 4x slower.
 - TRN2 has two physical HWDGE rings (SP + Activation). All large DMAs
   alternate between nc.sync and nc.scalar so the two rings run in parallel;
   a single ring serializes transfers (~206 GB/s) and dominates the span.
 - w0/w1 are host-interleaved into one [P, 2, 2*P] block per (fgroup, k) so
   each weight DMA is one 256 KB transfer with 2 KB partition lines.
 - One shared 8-bank PSUM pool: phase-1 holds 4 banks per f-group across its
   k-loop while the other 4 serve the previous group's eviction / phase 2.
"""

import os
import numpy as np

T, D, DFF, E, TOPK = 1024, 2048, 1024, 16, 6
NCORES, P = 8, 128
EPC = E // NCORES  # experts per core

# Set by kernel() after each run: BassKernelResults (exec_time_ns when traced).
LAST_RESULT = None

_PROGRAM_CACHE = {}


def _plan(NT):
    """Phase-1 token slices and f-group width, shared by host prep + builder.

    PSUM bank = 512 fp32, so TG <= 512 is one token slice; else two. FG is the
    number of DFF f-tiles accumulated per PSUM group (group = FG*len(slices)*2
    banks, kept at 4 so two groups pipeline in the 8-bank shared pool).
    """
    TG = NT * P
    if TG <= 512:
        slices = [(0, TG)]
        FG = 2
    else:
        h = (NT + 1) // 2 * P
        slices = [(0, h), (h, TG - h)]
        FG = 1
    return slices, FG


def _build_program(NT, d=D, dff=DFF, use_silu=True):
    """Build + compile the SPMD single-core Bass program for NT token tiles.

    use_silu=False decomposes silu into sigmoid+mul (CoreSim lacks Silu); the
    decomposed path computes ht = g*sigmoid(s0*g)*u, i.e. silu(s0*g)*u / s0.
    """
    import concourse.bacc as bacc
    import concourse.mybir as mybir
    import concourse.tile as tile

    f32 = mybir.dt.float32
    f32r = mybir.dt.float32r
    Silu = mybir.ActivationFunctionType.Silu

    TG = NT * P
    KD = d // P        # k-tiles over D (contraction of W0/W1 matmuls)
    KF = dff // P      # k-tiles over DFF (contraction of W2 matmul)
    DSW = min(512, d)  # output D slice width
    NDS = d // DSW     # output D slices
    slices, FG = _plan(NT)
    FGP = FG * P
    NFG = KF // FG

    nc = bacc.Bacc("TRN2", target_bir_lowering=False, debug=False)

    xt_d = nc.dram_tensor("xt", [EPC, d, TG], f32r, kind="ExternalInput").ap()
    w01_d = nc.dram_tensor("w01", [EPC, NFG, KD, P, 2, FGP], f32r,
                           kind="ExternalInput").ap()
    w2t_d = nc.dram_tensor("w2t", [EPC, dff, d], f32r, kind="ExternalInput").ap()
    s0_d = nc.dram_tensor("s0v", [EPC, P, 1], f32, kind="ExternalInput").ap()
    coef_d = nc.dram_tensor("coef", [EPC, NT, P, 1], f32, kind="ExternalInput").ap()
    y_d = nc.dram_tensor("y", [EPC, TG, d], f32, kind="ExternalOutput").ap()

    big_bufs = 2 if NT <= 4 else 1

    with tile.TileContext(nc) as tc:
        # Alternate large DMAs across the two physical HWDGE rings.
        rings = [nc.sync, nc.scalar]
        ring_state = [0]

        def ring():
            ring_state[0] ^= 1
            return rings[ring_state[0]]

        with (
            tc.tile_pool(name="xt", bufs=big_bufs) as xt_pool,
            tc.tile_pool(name="w01", bufs=16) as w01_pool,
            tc.tile_pool(name="w2", bufs=3) as w2_pool,
            tc.tile_pool(name="ht", bufs=big_bufs) as ht_pool,
            tc.tile_pool(name="act", bufs=6) as act_pool,
            tc.tile_pool(name="out", bufs=8) as out_pool,
            tc.tile_pool(name="sc", bufs=2) as sc_pool,
            tc.tile_pool(name="pgu", bufs=6, space="PSUM") as pgu_pool,
            tc.tile_pool(name="py", bufs=2, space="PSUM") as py_pool,
        ):
            for e in range(EPC):
                # --- inputs for this expert ---
                xt = xt_pool.tile([P, KD, TG], f32r, tag="xt")
                s0_sb = sc_pool.tile([P, 1], f32, tag="s0")
                nc.sync.dma_start(s0_sb[:], s0_d[e])
                ctiles = sc_pool.tile([P, NT], f32, tag="ctiles")
                for m in range(NT):
                    nc.sync.dma_start(ctiles[:, m:m + 1], coef_d[e, m])

                # --- phase 1: HT = silu(s0 * W0 xT) * (W1 xT), [DFF, TG] ---
                ht = ht_pool.tile([P, KF, TG], f32r, tag="ht")
                for fg in range(NFG):
                    psG = [[pgu_pool.tile([P, 512], f32, tag="pgu",
                                          name=f"psG_{e}_{fg}_{j}_{si}")
                            for si in range(len(slices))] for j in range(FG)]
                    psU = [[pgu_pool.tile([P, 512], f32, tag="pgu",
                                          name=f"psU_{e}_{fg}_{j}_{si}")
                            for si in range(len(slices))] for j in range(FG)]
                    for k in range(KD):
                        if fg == 0:
                            # interleave x loads with the first f-group's
                            # weight loads so matmul k can start as soon as
                            # its own xt/w01 slices land
                            ring().dma_start(xt[:, k, :],
                                             xt_d[e, k * P:(k + 1) * P, :])
                        w01b = w01_pool.tile([P, 2, FGP], f32r, tag="w01b")
                        ring().dma_start(w01b[:], w01_d[e, fg, k])
                        for j in range(FG):
                            for si, (t0, W) in enumerate(slices):
                                nc.tensor.matmul(
                                    psG[j][si][:, :W],
                                    w01b[:, 0, j * P:(j + 1) * P],
                                    xt[:, k, t0:t0 + W],
                                    start=(k == 0), stop=(k == KD - 1))
                                nc.tensor.matmul(
                                    psU[j][si][:, :W],
                                    w01b[:, 1, j * P:(j + 1) * P],
                                    xt[:, k, t0:t0 + W],
                                    start=(k == 0), stop=(k == KD - 1))
                    for j in range(FG):
                        f = fg * FG + j
                        for si, (t0, W) in enumerate(slices):
                            sig = act_pool.tile([P, 512], f32, tag="sig")
                            ht_f = ht[:, f, t0:t0 + W]  # f32r out: walrus
                            # requires fp32r-matmul operands written as fp32r
                            if use_silu:
                                nc.scalar.activation(
                                    sig[:, :W], psG[j][si][:, :W], Silu,
                                    scale=s0_sb[:])
                                nc.vector.tensor_mul(
                                    ht_f, sig[:, :W], psU[j][si][:, :W])
                            else:
                                nc.scalar.activation(
                                    sig[:, :W], psG[j][si][:, :W],
                                    mybir.ActivationFunctionType.Sigmoid,
                                    scale=s0_sb[:])
                                nc.vector.tensor_mul(
                                    ht_f, sig[:, :W], psU[j][si][:, :W])
                                nc.vector.tensor_mul(
                                    ht_f, ht_f, psG[j][si][:, :W])

                # --- phase 2: Y = HT^T @ W2^T, scaled per token ---
                for dsi in range(NDS):
                    w2b = w2_pool.tile([P, KF, DSW], f32r, tag="w2b")
                    # SWDGE (gpsimd) path: keeps phase-2 weights out of the
                    # HWDGE ring FIFOs (which phase-1 w01 loads monopolize);
                    # with bufs=3 the tiles allocate early enough that the
                    # slower SWDGE transfers still land before their matmuls
                    nc.gpsimd.dma_start(
                        w2b[:],
                        w2t_d[e, :, dsi * DSW:(dsi + 1) * DSW]
                        .rearrange("(k p) c -> p k c", p=P))
                    for m in range(NT):
                        psY = py_pool.tile([P, 512], f32, tag="py",
                                           name=f"psY_{e}_{dsi}_{m}")
                        for k in range(KF):
                            nc.tensor.matmul(
                                psY[:, :DSW], ht[:, k, m * P:(m + 1) * P],
                                w2b[:, k, :],
                                start=(k == 0), stop=(k == KF - 1))
                        ysb = out_pool.tile([P, DSW], f32, tag="ysb")
                        nc.vector.tensor_scalar_mul(
                            ysb[:], psY[:, :DSW], ctiles[:, m:m + 1])
                        ring().dma_start(
                            y_d[e, m * P:(m + 1) * P,
                                dsi * DSW:(dsi + 1) * DSW], ysb[:])

    nc.compile()
    return nc


def _prep_host(inputs):
    """Host-side dispatch: routing weights, per-expert token gather, layouts."""
    x = np.ascontiguousarray(np.asarray(inputs["x"], dtype=np.float32))
    w0 = np.asarray(inputs["w0"], dtype=np.float32)
    w1 = np.asarray(inputs["w1"], dtype=np.float32)
    w2 = np.asarray(inputs["w2"], dtype=np.float32)
    s0 = np.asarray(inputs["s0"], dtype=np.float32)
    s1 = np.asarray(inputs["s1"], dtype=np.float32)
    s2 = np.asarray(inputs["s2"], dtype=np.float32)
    se = np.asarray(inputs["selected_experts"]).astype(np.int64)
    rw = np.asarray(inputs["routing_weights"], dtype=np.float32)

    Tn, Dn = x.shape
    En, DFFn, _ = w0.shape
    KD = Dn // P
    KF = DFFn // P

    # combine weight per (expert, token): sum of routing weights over top-k slots
    cw = np.zeros((En, Tn), np.float32)
    cols = np.arange(Tn)
    for k in range(se.shape[1]):
        np.add.at(cw, (se[:, k], cols), rw[:, k])

    idx = [np.flatnonzero(cw[e] != 0.0) for e in range(En)]
    maxn = max(len(i) for i in idx)
    NT = max(2, -(-maxn // P))  # >=256 padded tokens keeps fp32r at full rate
    TG = NT * P
    slices, FG = _plan(NT)
    FGP = FG * P
    NFG = KF // FG

    xT = np.ascontiguousarray(x.T)  # [D, T]
    in_maps = []
    for c in range(NCORES):
        xt = np.zeros((EPC, Dn, TG), np.float32)
        coef = np.zeros((EPC, TG), np.float32)
        s0v = np.zeros((EPC, P, 1), np.float32)
        w01 = np.empty((EPC, NFG, KD, P, 2, FGP), np.float32)
        w2t = np.empty((EPC, DFFn, Dn), np.float32)
        for j in range(EPC):
            e = c * EPC + j
            ids = idx[e]
            xt[j, :, :len(ids)] = xT[:, ids]
            coef[j, :len(ids)] = s1[e] * s2[e] * cw[e, ids]
            s0v[j, :, 0] = s0[e]
            # [D, DFF] -> [NFG, KD, P, FGP] blocks, w0/w1 interleaved
            a = w0[e].T.reshape(KD, P, NFG, FGP).transpose(2, 0, 1, 3)
            b = w1[e].T.reshape(KD, P, NFG, FGP).transpose(2, 0, 1, 3)
            w01[j] = np.stack([a, b], axis=3)
            w2t[j] = w2[e].T
        in_maps.append({
            "xt": xt,
            "w01": w01,
            "w2t": w2t,
            "s0v": s0v,
            "coef": np.ascontiguousarray(coef.reshape(EPC, NT, P, 1)),
        })
    return in_maps, idx, NT, (Tn, Dn, DFFn)


def _combine(results, idx, shapes):
    """Unshard: scatter-add per-expert outputs into the dense [T, D] output."""
    Tn, Dn, _ = shapes
    out = np.zeros((Tn, Dn), np.float32)
    for c in range(NCORES):
        y = results[c]["y"]
        for j in range(EPC):
            e = c * EPC + j
            ids = idx[e]
            if len(ids):
                out[ids] += y[j, :len(ids), :]
    return out


def _ensure_axon_ntff_hook():
    """Provide antenv.axon_hooks if the image's antenv stub lacks it.

    concourse.bass_utils imports it unconditionally when BASS_TRACE/trace is
    set under axon; without this the run crashes. When libaxon_pjrt.so exposes
    the NRT-profile symbols we also install the real hook so NTFF profiling
    (HW exec times) works; otherwise tracing degrades to a warning.
    """
    import sys
    import types
    try:
        import antenv.axon_hooks  # noqa: F401
        return
    except ImportError:
        pass
    try:
        import antenv

        mod = types.ModuleType("antenv.axon_hooks")
        _state = {"hook": None}
        mod.set_axon_ntff_profile_hook = lambda h: _state.__setitem__("hook", h)
        mod.get_axon_ntff_profile_hook = lambda: _state["hook"]
        sys.modules["antenv.axon_hooks"] = mod
        antenv.axon_hooks = mod
        try:
            from trn_agent_boot.trn_boot import _ntff_profile_via_ctypes

            so = "/opt/axon/libaxon_pjrt.so"
            if os.path.exists(so):
                mod.set_axon_ntff_profile_hook(_ntff_profile_via_ctypes(so))
        except Exception:
            pass
    except Exception:
        pass


def kernel(**inputs) -> np.ndarray:
    global LAST_RESULT
    _ensure_axon_ntff_hook()
    from concourse.bass_utils import run_bass_kernel_spmd

    in_maps, idx, NT, shapes = _prep_host(inputs)

    key = (NT,) + shapes
    nc = _PROGRAM_CACHE.get(key)
    if nc is None:
        nc = _build_program(NT, d=shapes[1], dff=shapes[2])
        _PROGRAM_CACHE[key] = nc

    res = run_bass_kernel_spmd(nc, in_maps, core_ids=list(range(NCORES)))
    LAST_RESULT = res
    return _combine(res.results, idx, shapes)
